# revision 6
# baseline (speedup 1.0000x reference)
"""Trainium2 kernel for the autoregressive LSTM (fp8 DoubleRow + bf16 g-gate).

8 cores data-parallel over batch (BL=512 rows/core). Per core and step:
    z = W^T h + b    (W,b = R,bias at t=0; folded R+dw*kern, bias+db*kern
                      for t>0 — the autoregressive input folds into the
                      recurrence, so no per-step dense feedback is needed)
    i,f,o = sigmoid(z...), g = tanh(z_g); c = f*c + i*g; h = o*tanh(c)
    pred_t = h @ dw + db -> out[t]

Precision design (validated in sim_fp8.py):
  - i/f/o gate columns: fp8e4 weights x fp8e4 h via DoubleRow matmuls
    (2x bf16 FLOP rate). Quantization noise ~4% rms on z, but those gates
    pass it through a sigmoid with slope <= 1/4.
  - g gate columns: plain bf16 x bf16-h matmuls. tanh has slope 1, so the
    g gate carries ~16x more noise variance per unit z-noise than the
    sigmoid gates — cleaning just these 8 of 32 j-tiles removes ~78% of
    the injected error at 1/4 of the dual-precision cost.
  - c state bf16, gates bf16, pred from the bf16 h copy.
Host does all quantization/layout; the device runs only the scan.

Scheduling: the previous step's dense pred (8 small bf16 matmuls, inputs
long ready) is emitted at the START of the next step so the PE chews on it
while ACT/DVE finish the previous step's last h chunks — without this the
first DoubleRow chains of each step stall ~2us waiting for h8[6:8].
Measured: 4.79 ms HW exec (baseline bf16 kernel: 8.52 ms), rel err 1.37e-2.
"""

import sys

sys.path.insert(0, "/opt/trn_rl_repo")

import numpy as np
import ml_dtypes

import concourse.bass as bass
import concourse.tile as tile
from concourse import bacc, mybir
from concourse.bass_utils import run_bass_kernel_spmd

B = 4096
FEAT = 512
U = 1024
J = 4 * U
T_STEPS = 128
N_CORES = 8
BL = B // N_CORES
KC = U // 128       # 8 contraction chunks
JT = J // 128       # 32 j tiles
JJ8 = 3 * U         # fp8 (i,f,o) column count

F32 = mybir.dt.float32
BF16 = mybir.dt.bfloat16
F8 = mybir.dt.float8e4
DR = mybir.MatmulPerfMode.DoubleRow
AF = mybir.ActivationFunctionType
OP = mybir.AluOpType

BF = ml_dtypes.bfloat16
NPF8 = mybir.dt.np(F8)

FP8_GATES = (0, 1, 3)           # i, f, o
GATE_FUNCS = [AF.Sigmoid, AF.Sigmoid, AF.Tanh, AF.Sigmoid]


def build_program(t_steps: int, sc0: float, sc1: float, s_h: float):
    nc = bacc.Bacc(None, target_bir_lowering=False)

    w8d = nc.declare_dram_parameter("w8", [128, KC, JJ8], F8, isOutput=False)
    r8d = nc.declare_dram_parameter("r8", [128, KC, JJ8], F8, isOutput=False)
    wgd = nc.declare_dram_parameter("wg", [128, KC, U], BF16, isOutput=False)
    rgd = nc.declare_dram_parameter("rg", [128, KC, U], BF16, isOutput=False)
    h80 = nc.declare_dram_parameter("h80", [128, KC, BL], F8, isOutput=False)
    hbf0 = nc.declare_dram_parameter("hbf0", [128, KC, BL], BF16, isOutput=False)
    c0 = nc.declare_dram_parameter("c0", [128, KC, BL], BF16, isOutput=False)
    bias0 = nc.declare_dram_parameter("bias0", [128, JT], F32, isOutput=False)
    bias1 = nc.declare_dram_parameter("bias1", [128, JT], F32, isOutput=False)
    dws = nc.declare_dram_parameter("dws", [128, KC], BF16, isOutput=False)
    db = nc.declare_dram_parameter("db", [1], F32, isOutput=False)
    out = nc.declare_dram_parameter("out", [t_steps, BL], F32, isOutput=True)

    with tile.TileContext(nc) as tc:
        with (
            tc.tile_pool(name="persist", bufs=1) as persist,
            tc.tile_pool(name="zpsum", bufs=6, space="PSUM") as zpsum,
            tc.tile_pool(name="ppsum", bufs=2, space="PSUM") as ppsum,
        ):
            W8 = persist.tile([128, KC, JJ8], F8, tag="w8")
            R8 = persist.tile([128, KC, JJ8], F8, tag="r8")
            WG = persist.tile([128, KC, U], BF16, tag="wg")
            RG = persist.tile([128, KC, U], BF16, tag="rg")
            h8A = persist.tile([128, KC, BL], F8, tag="h8a")
            h8B = persist.tile([128, KC, BL], F8, tag="h8b")
            hbA = persist.tile([128, KC, BL], BF16, tag="hba")
            hbB = persist.tile([128, KC, BL], BF16, tag="hbb")
            cT = persist.tile([128, KC, BL], BF16, tag="c")
            bT0 = persist.tile([128, JT], F32, tag="bias0")
            bT1 = persist.tile([128, JT], F32, tag="bias1")
            dw_sb = persist.tile([128, KC], BF16, tag="dws")
            db_sb = persist.tile([1, 1], F32, tag="dbsb")
            zb = persist.tile([128, 1], F32, tag="zb")

            nc.sync.dma_start(out=W8[:], in_=w8d[:, :, :])
            nc.sync.dma_start(out=R8[:], in_=r8d[:, :, :])
            nc.sync.dma_start(out=WG[:], in_=wgd[:, :, :])
            nc.sync.dma_start(out=RG[:], in_=rgd[:, :, :])
            nc.sync.dma_start(out=h8A[:], in_=h80[:, :, :])
            nc.sync.dma_start(out=hbA[:], in_=hbf0[:, :, :])
            nc.sync.dma_start(out=cT[:], in_=c0[:, :, :])
            nc.sync.dma_start(out=bT0[:], in_=bias0[:, :])
            nc.sync.dma_start(out=bT1[:], in_=bias1[:, :])
            nc.sync.dma_start(out=dw_sb[:], in_=dws[:, :])
            nc.sync.dma_start(out=db_sb[:], in_=db[:].to_broadcast((1, 1)))
            nc.vector.memset(zb[:], 0.0)

            h8bufs = [h8A, h8B]
            hbbufs = [hbA, hbB]

            with (
                tc.tile_pool(name="gates", bufs=12) as gates_pool,
                tc.tile_pool(name="tmps", bufs=4) as tmp_pool,
                tc.tile_pool(name="ths", bufs=2) as th_pool,
                tc.tile_pool(name="prows", bufs=2) as prow_pool,
            ):
                def emit_pred(t_idx, hb):
                    pp = ppsum.tile([1, BL], F32, tag="pp")
                    for k in range(KC):
                        nc.tensor.matmul(
                            pp[:], dw_sb[:, k:k + 1], hb[:, k, :],
                            start=(k == 0), stop=(k == KC - 1),
                        )
                    prow = prow_pool.tile([1, BL], F32, tag="prow")
                    nc.scalar.activation(
                        out=prow[:], in_=pp[:], func=AF.Identity,
                        bias=db_sb[0:1, 0:1],
                    )
                    nc.sync.dma_start(out=out[t_idx:t_idx + 1, :], in_=prow[:])

                for t in range(t_steps):
                    W8s, WGs = (R8, RG) if t == 0 else (W8, WG)
                    bTs = bT0 if t == 0 else bT1
                    sc = sc0 if t == 0 else sc1
                    h8cur = h8bufs[t % 2]
                    h8nxt = h8bufs[(t + 1) % 2]
                    hbcur = hbbufs[t % 2]
                    hbnxt = hbbufs[(t + 1) % 2]

                    if t > 0:
                        # previous step's dense pred: its inputs are long
                        # ready; runs while ACT/DVE finish step t-1's tail
                        emit_pred(t - 1, hbcur)
                    for k in range(KC):
                        gt = []
                        for g in range(4):
                            jt = g * KC + k
                            zp = zpsum.tile([128, BL], F32, tag="zp")
                            if g == 2:  # tanh gate: clean bf16 path
                                for kk in range(KC):
                                    nc.tensor.matmul(
                                        zp[:],
                                        WGs[:, kk, k * 128:(k + 1) * 128],
                                        hbcur[:, kk, :],
                                        start=(kk == 0), stop=(kk == KC - 1),
                                    )
                                gsc = 1.0
                            else:
                                gi = FP8_GATES.index(g)
                                jc = (gi * KC + k) * 128
                                for kp in range(KC // 2):
                                    nc.tensor.matmul(
                                        zp[:],
                                        W8s[:, 2 * kp:2 * kp + 2, jc:jc + 128],
                                        h8cur[:, 2 * kp:2 * kp + 2, :],
                                        start=(kp == 0),
                                        stop=(kp == KC // 2 - 1),
                                        perf_mode=DR,
                                    )
                                gsc = sc
                            gtile = gates_pool.tile([128, BL], BF16, tag="gate")
                            nc.scalar.activation(
                                out=gtile[:], in_=zp[:], func=GATE_FUNCS[g],
                                bias=bTs[:, jt:jt + 1], scale=gsc,
                            )
                            gt.append(gtile)
                        gi_, gf_, gg_, go_ = gt
                        ig = tmp_pool.tile([128, BL], BF16, tag="tmp")
                        nc.vector.tensor_tensor(ig[:], gi_[:], gg_[:], OP.mult)
                        fc = tmp_pool.tile([128, BL], BF16, tag="tmp")
                        nc.vector.tensor_tensor(fc[:], gf_[:], cT[:, k, :], OP.mult)
                        nc.vector.tensor_tensor(cT[:, k, :], ig[:], fc[:], OP.add)
                        th = th_pool.tile([128, BL], BF16, tag="th")
                        nc.scalar.activation(
                            out=th[:], in_=cT[:, k, :], func=AF.Tanh,
                            bias=zb[:, 0:1],
                        )
                        nc.vector.tensor_tensor(hbnxt[:, k, :], go_[:], th[:], OP.mult)
                        nc.vector.scalar_tensor_tensor(
                            out=h8nxt[:, k, :], in0=go_[:], scalar=s_h,
                            in1=th[:], op0=OP.mult, op1=OP.mult,
                        )

                emit_pred(t_steps - 1, hbbufs[t_steps % 2])

    nc.compile()
    return nc


_PROGRAM_CACHE = {}


def _q8(x, s):
    return np.clip(np.asarray(x, np.float32) * s, -240.0, 240.0).astype(NPF8)


def _chunked(x_uj):
    """[U, X] -> [128, KC, X] with u = kc*128 + p."""
    Xw = x_uj.shape[1]
    return np.ascontiguousarray(x_uj.reshape(KC, 128, Xw).transpose(1, 0, 2))


def _ifo(x):
    """[U, J] -> i,f,o columns [U, 3U]."""
    return np.concatenate([x[:, 0:U], x[:, U:2 * U], x[:, 3 * U:4 * U]], axis=1)


def prepare_inputs(inputs):
    feats = np.asarray(inputs["features"], dtype=np.float32)
    R = np.asarray(inputs["recurrent_kernel"], dtype=np.float32)
    kern = np.asarray(inputs["kernel"], dtype=np.float32).reshape(1, J)
    bias = np.asarray(inputs["bias"], dtype=np.float32).reshape(J)
    dw = np.asarray(inputs["dense_w"], dtype=np.float32).reshape(U, 1)
    dbv = np.asarray(inputs["dense_b"], dtype=np.float32).reshape(1)

    Wp = R + dw @ kern
    bias1 = bias + dbv[0] * kern[0]
    s_w = 224.0 / max(np.abs(_ifo(Wp)).max(), np.abs(_ifo(R)).max())
    am0 = np.abs(feats).max()
    s0 = min(32.0, 224.0 / max(am0, 1e-6))
    s_h = 128.0

    parts = {
        "w8": _chunked(_q8(_ifo(Wp), s_w)),
        "r8": _chunked(_q8(_ifo(R), s_w)),
        "wg": _chunked(Wp[:, 2 * U:3 * U].astype(BF)),
        "rg": _chunked(R[:, 2 * U:3 * U].astype(BF)),
        "bias0": np.ascontiguousarray(bias.reshape(JT, 128).T).astype(np.float32),
        "bias1": np.ascontiguousarray(bias1.reshape(JT, 128).T).astype(np.float32),
        "dws": np.ascontiguousarray(dw[:, 0].reshape(KC, 128).T).astype(BF),
        "db": dbv,
    }
    in_maps = []
    for i in range(N_CORES):
        f = feats[i * BL:(i + 1) * BL]
        h0 = np.concatenate([f, f], axis=1).T        # [U, BL]
        h0c = _chunked(h0)
        m = dict(parts)
        m["h80"] = _q8(h0c, s0)
        m["hbf0"] = h0c.astype(BF)
        m["c0"] = h0c.astype(BF)
        in_maps.append(m)
    return in_maps, s_w, s0, s_h


def run(inputs: dict, t_steps: int = T_STEPS, trace: bool = False):
    in_maps, s_w, s0, s_h = prepare_inputs(inputs)
    key = (t_steps, round(float(s_w), 6), round(float(s0), 6))
    if key not in _PROGRAM_CACHE:
        _PROGRAM_CACHE[key] = build_program(
            t_steps, 1.0 / (s_w * s0), 1.0 / (s_w * s_h), s_h)
    nc = _PROGRAM_CACHE[key]

    res = run_bass_kernel_spmd(
        nc, in_maps, core_ids=list(range(N_CORES)), trace=trace
    )
    outs = [np.asarray(res.results[i]["out"]) for i in range(N_CORES)]
    full = np.concatenate([o.T for o in outs], axis=0)[:, :, None]
    return full.astype(np.float32), res


def kernel(**inputs) -> np.ndarray:
    out, _ = run(inputs, t_steps=T_STEPS, trace=False)
    return out
